# revision 28
# baseline (speedup 1.0000x reference)
"""BDH linear-attention Trainium2 kernel, data-parallel over batch on 8 cores.

fp8 DoubleRow edition. Per-core program (core b handles batch b):
  A. LayerNorm (bn_stats) -> xn bf16, residue-logit accumulation, PE transpose
     -> xnT8 (fp8, [d 128][2][tok] DoubleRow k-tile layout, scaled by s_x)
  B. k/v projections as fp8 DR matmuls vs streamed packed w_in8 (k of 2 heads
     | v of 2 heads per pair block); hub map khat ~ relu(k)^1.5 via
     ACT Relu (dequant folded in scale) + DVE (r+c2)*r -> khat8;
     v8 via Pool scale-copy; per-head state stateT[e,d] = v8^T khat8 (fp8 DR),
     k-normalization via ones-column z + reciprocal broadcast; bf16 states +
     write gates -> DRAM -> AllReduce(add) over 8 cores (two halves).
  C. q projection (fp8 DR) + hub map -> qhatT8 [s][tok]; absb gate scalars.
  D. mt8 = s_m*(memT + ba_h*stT) (fp8, e'-chunk layout, zero-padded to 4
     chunks), W'_h = mt8^T @ w_out8 (fp8 DR), evac * read-scale -> Wp8.
  E. out = sum_s qhatT8^T @ Wp8 (fp8 DR, accumulated over all 24 s-chunks),
     blend out = x + residue*(y*c_deq - x), DMA out.

kernel(**inputs) quantizes/packs the weights host-side (fp8 e4m3 with static
power-of-two-ish scales), shards batch across 8 cores, reassembles the output.
"""
import numpy as np
import ml_dtypes

import concourse.mybir as mybir
import concourse.tile as tile
from concourse import bacc
from concourse.masks import make_identity
from concourse.bass_utils import run_bass_kernel_spmd

F32 = mybir.dt.float32
F32R = mybir.dt.float32r
BF16 = mybir.dt.bfloat16
F8 = mybir.dt.float8e4
AF = mybir.ActivationFunctionType
OP = mybir.AluOpType
PM = mybir.MatmulPerfMode

B, N, D, H = 8, 1024, 768, 8
S = 3072
HD = 384
NT = N // 128        # 8 token tiles
EPS = 1e-6
LN_EPS = 1e-5
PERSIST = 0.95
N_CORES = 8
CC_HALF = 4 * HD * HD
CC_LEN_A = CC_HALF + 8
CC_LEN_B = CC_HALF

# ---- quantization scales (host <-> device contract) ----
S_X = 4.0            # xn fp8 scale
S_W = 32.0           # w_in fp8 scale
S_V = 16.0           # v fp8 scale
S_M = 64.0           # mt fp8 scale (memT prescaled on host)
S_P = 256.0          # Wp fp8 scale
S_WO = 32.0          # w_out fp8 scale
# hub poly: t^1.5 ~ t*(A_F*t + B_F) on t in [0, ~3.2]
A_F = 0.49630892
B_F = 0.48600504
S_HUB = 32.0         # effective khat8/qhat8 scale
GAM = float(np.sqrt(A_F * S_HUB))          # r = GAM * t
C1 = GAM / (S_X * S_W)                     # ACT relu scale on psum
C2 = float((B_F / A_F) * GAM)              # khat8 = (r + C2) * r
C_V = S_V / (S_X * S_W)                    # v evac scale
C_DEQ = 1.0 / (S_HUB * S_P)                # out psum dequant
C_GATE = 1.0 / (N * S_X * S_W)             # gate logit scale
E_WP = PERSIST * S_P / (S_M * S_WO)        # * read_h -> Wp evac scale
E_BA = S_M * (1.0 - PERSIST) / (N_CORES * N_CORES * S_V * PERSIST)
Z_EPS = S_HUB * 1e-6

DEBUG_DUMPS = False

KVB = 1536           # cols per kv pair block [k0|k1|v0|v1]
QOFF = 4 * KVB       # 6144: q block start in packed w8 image
GOFF = QOFF + S      # 9216: gates block
WCOLS = GOFF + 16    # 9232


def build_program(ln_trivial, b_in_zero, b_out_zero, reps=1, single_core=False):
    nc = bacc.Bacc("TRN2", target_bir_lowering=False, debug=False,
                   num_devices=1 if single_core else N_CORES)

    x_d = nc.dram_tensor("x", [N, D], F32, kind="ExternalInput")
    w8_d = nc.dram_tensor("w8", [3, 128, 2, WCOLS], F8, kind="ExternalInput")
    wo8_d = nc.dram_tensor("wo8", [H, 128, 4, D], F8, kind="ExternalInput")
    memT_d = nc.dram_tensor("memT", [H, HD, HD], BF16, kind="ExternalInput")
    gb_d = nc.dram_tensor("gb", [16], F32, kind="ExternalInput")
    w_res_d = nc.dram_tensor("w_res", [D, 1], F32, kind="ExternalInput")
    b_res_d = nc.dram_tensor("b_res", [1], F32, kind="ExternalInput")
    out_d = nc.dram_tensor("out", [N, D], F32, kind="ExternalOutput")
    if DEBUG_DUMPS:
        dbg = {
            "xnT8_0": nc.dram_tensor("dbg_xnT8_0", [128, 2, N], F8,
                                     kind="ExternalOutput"),
            "khat00": nc.dram_tensor("dbg_khat00", [128, 2, 768], F8,
                                     kind="ExternalOutput"),
            "v800": nc.dram_tensor("dbg_v800", [128, 2, 768], F8,
                                   kind="ExternalOutput"),
            "z0": nc.dram_tensor("dbg_z0", [1, HD], F32,
                                 kind="ExternalOutput"),
            "st00": nc.dram_tensor("dbg_st00", [128, HD], BF16,
                                   kind="ExternalOutput"),
            "qh0": nc.dram_tensor("dbg_qh0", [128, 2, N], F8,
                                  kind="ExternalOutput"),
            "wp0": nc.dram_tensor("dbg_wp0", [128, 2, D], F8,
                                  kind="ExternalOutput"),
            "mt80": nc.dram_tensor("dbg_mt80", [128, 4, HD], F8,
                                   kind="ExternalOutput"),
            "gates": nc.dram_tensor("dbg_gates", [1, 16], F32,
                                    kind="ExternalOutput"),
            "absb": nc.dram_tensor("dbg_absb", [128, 16], F32,
                                   kind="ExternalOutput"),
            "residue": nc.dram_tensor("dbg_residue", [128, NT], F32,
                                      kind="ExternalOutput"),
            "stbf0": nc.dram_tensor("dbg_stbf0", [128, 3, HD], BF16,
                                    kind="ExternalOutput"),
        }
    if not ln_trivial:
        ln_g_d = nc.dram_tensor("ln_g", [D], F32, kind="ExternalInput")
        ln_b_d = nc.dram_tensor("ln_b", [D], F32, kind="ExternalInput")
    if not b_in_zero:
        b_in_d = nc.dram_tensor("b_in", [3 * S], F32, kind="ExternalInput")
    if not b_out_zero:
        b_out_d = nc.dram_tensor("b_out", [D], F32, kind="ExternalInput")

    with tile.TileContext(nc) as tc:
      for rep in range(reps):
        P = lambda nm: f"{nm}_r{rep}"
        with (
            tc.tile_pool(name=P("const"), bufs=1) as const,
            tc.tile_pool(name=P("persist"), bufs=1) as persist,
            tc.tile_pool(name=P("ccdram"), bufs=1, space="DRAM") as ccdram,
        ):
            ident_bf = const.tile([128, 128], BF16)
            identf = const.tile([128, 128], F32)
            make_identity(nc, identf[:])
            nc.vector.tensor_copy(ident_bf[:], identf[:])
            lneps_col = const.tile([128, 1], F32)
            nc.vector.memset(lneps_col[:], LN_EPS)
            ones2b = const.tile([128, 2, 32], BF16)
            nc.vector.memset(ones2b[:], 1.0)
            ones8 = const.tile([128, 2, 32], F8)
            nc.vector.tensor_copy(ones8[:], ones2b[:])
            ones_row_f = const.tile([1, 128], F32)
            nc.vector.memset(ones_row_f[:], 1.0)
            ones_row_r = const.tile([1, 128], F32R)
            nc.vector.tensor_copy(ones_row_r[:], ones_row_f[:])
            onesb_col = const.tile([128, 1], BF16)
            nc.vector.memset(onesb_col[:], 1.0)
            wres_b = const.tile([128, D], F32)
            nc.sync.dma_start(wres_b[:], w_res_d.ap().opt().partition_broadcast(128))
            bres_b = const.tile([128, 1], F32)
            nc.sync.dma_start(bres_b[:], b_res_d.ap().partition_broadcast(128))
            gbias = const.tile([1, 16], F32)
            nc.sync.dma_start(gbias[:], gb_d.ap().partition_broadcast(1))
            if not ln_trivial:
                lng_b = const.tile([128, D], F32)
                nc.sync.dma_start(lng_b[:], ln_g_d.ap().partition_broadcast(128))
                lnb_b = const.tile([128, D], F32)
                nc.sync.dma_start(lnb_b[:], ln_b_d.ap().partition_broadcast(128))
            if not b_in_zero:
                bkv_bc = {}
                for hp in range(4):
                    bk = const.tile([128, 768], F32, name=f"bk{rep}_{hp}")
                    bv = const.tile([128, 768], F32, name=f"bv{rep}_{hp}")
                    c0 = S + hp * 2 * HD
                    v0 = 2 * S + hp * 2 * HD
                    nc.sync.dma_start(
                        bk[:], b_in_d.ap()[c0:c0 + 768].partition_broadcast(128))
                    nc.sync.dma_start(
                        bv[:], b_in_d.ap()[v0:v0 + 768].partition_broadcast(128))
                    bkv_bc[hp] = (bk, bv)
                # per-chunk q bias as per-partition scalar [128, 24]
                bq_sc = const.tile([128, S // 128], F32, name=f"bqsc{rep}")
                nc.sync.dma_start(
                    bq_sc[:],
                    b_in_d.ap()[0:S].rearrange("(q p) -> p q", p=128))

            # persistent activation tensors
            xnT8 = [persist.tile([128, 2, N], F8, name=f"xnT8_{rep}_{j}")
                    for j in range(3)]
            qhatT8 = [persist.tile([128, 2, N], F8, name=f"qhT8_{rep}_{j}")
                      for j in range(12)]
            Wp8 = [persist.tile([128, 2, D], F8, name=f"Wp8_{rep}_{j}")
                   for j in range(12)]
            rlog_all = persist.tile([128, NT], F32)
            gates_sb = persist.tile([1, 16], F32)
            residue = persist.tile([128, NT], F32)
            rs_col = persist.tile([128, NT], F32)
            onemr = persist.tile([128, NT], F32)
            absb = persist.tile([128, 16], F32)

            cc_in_a = ccdram.tile([CC_LEN_A], BF16)
            cc_in_b = ccdram.tile([CC_LEN_B], BF16)
            cc_out_a = ccdram.tile([CC_LEN_A], BF16,
                                   addr_space="Local" if single_core else "Shared")
            cc_out_b = ccdram.tile([CC_LEN_B], BF16,
                                   addr_space="Local" if single_core else "Shared")

            # q weights (+gates), resident: [3][128, 2, S+16]
            w8q = [persist.tile([128, 2, S + 16], F8, name=f"w8q_{rep}_{j}")
                   for j in range(3)]

            # ============== phases A+B (kv + states) ==============
            with (
                tc.tile_pool(name=P("wkv"), bufs=6) as wkv,
                tc.tile_pool(name=P("kvp"), bufs=16) as kvp,
                tc.tile_pool(name=P("hubp"), bufs=4) as hubp,
                tc.tile_pool(name=P("ps_k"), bufs=2, space="PSUM") as ps_k,
                tc.tile_pool(name=P("ps_v"), bufs=1, space="PSUM") as ps_v,
            ):
                kv_w = {}
                kv_tiles = {}

                def kv_weights(hp):
                    wt = [wkv.tile([128, 2, KVB], F8, tag="wkv",
                                   name=f"wkv_{rep}_{hp}_{j}") for j in range(3)]
                    for j in range(3):
                        nc.sync.dma_start(
                            wt[j][:], w8_d[j][:, :, hp * KVB:(hp + 1) * KVB])
                    kv_w[hp] = wt

                def kv_mm(hp, t):
                    wt = kv_w[hp]
                    psk = ps_k.tile([128, 768], F32, tag="psk")
                    psv = ps_v.tile([128, 768], F32, tag="psv")
                    for j in range(3):
                        st_ap = xnT8[j][:, :, t * 128:(t + 1) * 128]
                        nc.tensor.matmul(psk[:, 0:512], st_ap,
                                         wt[j][:, :, 0:512],
                                         start=(j == 0), stop=(j == 2),
                                         perf_mode=PM.DoubleRow)
                        nc.tensor.matmul(psk[:, 512:768], st_ap,
                                         wt[j][:, :, 512:768],
                                         start=(j == 0), stop=(j == 2),
                                         perf_mode=PM.DoubleRow)
                        nc.tensor.matmul(psv[:, 0:512], st_ap,
                                         wt[j][:, :, 768:1280],
                                         start=(j == 0), stop=(j == 2),
                                         perf_mode=PM.DoubleRow)
                        nc.tensor.matmul(psv[:, 512:768], st_ap,
                                         wt[j][:, :, 1280:1536],
                                         start=(j == 0), stop=(j == 2),
                                         perf_mode=PM.DoubleRow)
                    if not b_in_zero:
                        nc.vector.tensor_add(psk[:], psk[:], bkv_bc[hp][0][:])
                        nc.vector.tensor_add(psv[:], psv[:], bkv_bc[hp][1][:])
                    r, i = t // 2, t % 2
                    key = (hp, r)
                    if key not in kv_tiles:
                        kv_tiles[key] = (
                            kvp.tile([128, 2, 768], F8, tag="khat",
                                     name=f"khat_{rep}_{hp}_{r}"),
                            kvp.tile([128, 2, 768], F8, tag="v8",
                                     name=f"v8_{rep}_{hp}_{r}"),
                        )
                    khat8, v8 = kv_tiles[key]
                    dump_kv = DEBUG_DUMPS and hp == 0 and t == 1
                    # hub map on k (stt on Pool: SBUF-only, frees DVE)
                    rbf = hubp.tile([128, 768], BF16, tag="rbf")
                    nc.scalar.activation(rbf[:], psk[:], AF.Relu, scale=C1)
                    nc.vector.scalar_tensor_tensor(
                        khat8[:, i, :], rbf[:], C2, rbf[:], OP.add, OP.mult)
                    # v evac (GPSIMD can't read PSUM; ACT)
                    nc.scalar.activation(v8[:, i, :], psv[:], AF.Copy,
                                         scale=C_V)
                    if dump_kv:
                        nc.sync.dma_start(dbg["khat00"].ap(), khat8[:])
                        nc.sync.dma_start(dbg["v800"].ap(), v8[:])
                        nc.sync.dma_start(dbg["xnT8_0"].ap(), xnT8[0][:])

                # ---------------- A: LN + transpose (+ pair-0 kv)
                kv_weights(0)
                kv_weights(1)
                with (
                    tc.tile_pool(name=P("xp"), bufs=8) as xp,
                    tc.tile_pool(name=P("xnbf"), bufs=3) as xnbf,
                    tc.tile_pool(name=P("lnp"), bufs=3) as lnp,
                    tc.tile_pool(name=P("ps_tp"), bufs=2, space="PSUM") as ps_tp,
                ):
                    x_tiles = []
                    for t in range(NT):
                        x_sb = xp.tile([128, D], F32, tag="x",
                                       name=f"xsb_{rep}_{t}")
                        nc.scalar.dma_start(x_sb[:], x_d[t * 128:(t + 1) * 128, :])
                        x_tiles.append(x_sb)
                    for j in range(3):
                        nc.sync.dma_start(w8q[j][:], w8_d[j][:, :, QOFF:WCOLS])
                    for t in range(NT):
                        x_sb = x_tiles[t]
                        stats = lnp.tile([128, 3, 6], F32, tag="stats")
                        for g in range(3):
                            nc.vector.bn_stats(stats[:, g, :],
                                               x_sb[:, g * 256:(g + 1) * 256])
                        mv = lnp.tile([128, 2], F32, tag="mv")
                        nc.vector.bn_aggr(mv[:], stats[:])
                        sq = lnp.tile([128, 1], F32, tag="sq")
                        nc.scalar.activation(sq[:], mv[:, 1:2], AF.Sqrt,
                                             bias=lneps_col[:], scale=1.0)
                        rstd = lnp.tile([128, 1], F32, tag="rstd")
                        nc.vector.reciprocal(rstd[:], sq[:])
                        xn = xnbf.tile([128, D], BF16, tag="xn")
                        nc.vector.tensor_scalar(xn[:], x_sb[:], mv[:, 0:1],
                                                rstd[:], OP.subtract, OP.mult)
                        if not ln_trivial:
                            nc.vector.tensor_mul(xn[:], xn[:], lng_b[:])
                            nc.vector.tensor_add(xn[:], xn[:], lnb_b[:])
                        # residue logit
                        scr = lnp.tile([128, D], BF16, tag="scr")
                        nc.vector.scalar_tensor_tensor(
                            scr[:], xn[:], 0.0, wres_b[:], OP.add, OP.mult,
                            accum_out=rlog_all[:, t:t + 1])
                        # transpose + fp8 evac
                        tp = ps_tp.tile([128, 768], BF16, tag="tp")
                        for c in range(6):
                            nc.tensor.transpose(tp[:, c * 128:(c + 1) * 128],
                                                xn[:, c * 128:(c + 1) * 128],
                                                ident_bf[:])
                        for j in range(3):
                            nc.vector.tensor_scalar_mul(
                                xnT8[j][:, :, t * 128:(t + 1) * 128],
                                tp[:, 2 * j * 128:(2 * j + 2) * 128], S_X)
                        kv_mm(0, t)

                with (
                    tc.tile_pool(name=P("stp"), bufs=6) as stp,
                    tc.tile_pool(name=P("ps_sm"), bufs=2, space="PSUM") as ps_sm,
                ):
                    # gates (sigmoid table era begins here)
                    gt = ps_sm.tile([128, HD], F32, tag="sm")
                    gps = gt[:, 0:16]
                    for j in range(3):
                        for t in range(NT):
                            nc.tensor.matmul(
                                gps, xnT8[j][:, :, t * 128:(t + 1) * 128],
                                w8q[j][:, :, S:S + 16],
                                start=(j == 0 and t == 0),
                                stop=(j == 2 and t == 7),
                                perf_mode=PM.DoubleRow)
                    gsb = stp.tile([128, 16], BF16, tag="gsb")
                    nc.vector.tensor_copy(gsb[:], gps)
                    gt2 = ps_sm.tile([128, HD], F32, tag="sm")
                    gps2 = gt2[0:1, 0:16]
                    nc.tensor.matmul(gps2, onesb_col[:], gsb[:])
                    glog = stp.tile([1, 16], F32, tag="glog")
                    nc.vector.scalar_tensor_tensor(glog[:], gps2, C_GATE,
                                                   gbias[:], OP.mult, OP.add)
                    nc.scalar.activation(gates_sb[:], glog[:], AF.Sigmoid)
                    wr16 = stp.tile([1, 8], BF16, tag="wr16")
                    nc.vector.tensor_copy(wr16[:], gates_sb[:, 8:16])
                    nc.sync.dma_start(cc_in_a[CC_HALF:CC_HALF + 8],
                                      wr16[:].opt())

                    def state_part(hp):
                        khat8, v8 = {}, {}
                        for r in range(4):
                            khat8[r], v8[r] = kv_tiles.pop((hp, r))
                        for hh in range(2):
                            h = 2 * hp + hh
                            off = hh * 384
                            zt = ps_sm.tile([128, HD], F32, tag="sm")
                            zps = zt[0:32, :]
                            for r in range(4):
                                nc.tensor.matmul(
                                    zps, ones8[:],
                                    khat8[r][:, :, off:off + 384],
                                    start=(r == 0), stop=(r == 3),
                                    perf_mode=PM.DoubleRow)
                            zrow = stp.tile([1, HD], F32, tag="zrow")
                            nc.vector.tensor_scalar_add(zrow[:], zt[0:1, :],
                                                        Z_EPS)
                            if DEBUG_DUMPS and h == 0:
                                nc.sync.dma_start(dbg["z0"].ap(), zrow[:])
                            zrc = stp.tile([1, HD], F32, tag="zrc")
                            nc.vector.reciprocal(zrc[:], zrow[:])
                            zrec = stp.tile([1, HD], F32R, tag="zrec")
                            nc.vector.tensor_copy(zrec[:], zrc[:])
                            rbp = ps_sm.tile([128, HD], F32, tag="sm")
                            nc.tensor.matmul(rbp[:], ones_row_r[:], zrec[:])
                            rb_sb = stp.tile([128, HD], F32, tag="rbsb")
                            nc.scalar.copy(rb_sb[:], rbp[:])
                            cc_t = cc_in_a if h < 4 else cc_in_b
                            for ec in range(3):
                                pst = ps_sm.tile([128, HD], F32, tag="sm")
                                for r in range(4):
                                    nc.tensor.matmul(
                                        pst[:],
                                        v8[r][:, :,
                                              off + ec * 128:off + ec * 128 + 128],
                                        khat8[r][:, :, off:off + 384],
                                        start=(r == 0), stop=(r == 3),
                                        perf_mode=PM.DoubleRow)
                                st_sb = stp.tile([128, HD], BF16, tag="stsb")
                                nc.vector.tensor_mul(st_sb[:], pst[:], rb_sb[:])
                                if DEBUG_DUMPS and h == 0 and ec == 0:
                                    nc.sync.dma_start(dbg["st00"].ap(),
                                                      st_sb[:])
                                base = (h % 4) * HD * HD + ec * 128 * HD
                                nc.sync.dma_start(
                                    cc_t[base:base + 128 * HD]
                                    .rearrange("(p f) -> p f", p=128),
                                    st_sb[:])
                        if hp == 1 or hp == 3:
                            cin = cc_in_a if hp == 1 else cc_in_b
                            cout = cc_out_a if hp == 1 else cc_out_b
                            clen = CC_LEN_A if hp == 1 else CC_LEN_B
                            if single_core:
                                nmain = (clen // 9216) * 9216
                                nc.sync.dma_start(
                                    cout[0:nmain]
                                    .rearrange("(p f) -> p f", p=128),
                                    cin[0:nmain]
                                    .rearrange("(p f) -> p f", p=128))
                                if clen > nmain:
                                    nc.sync.dma_start(cout[nmain:clen],
                                                      cin[nmain:clen])
                            else:
                                nc.gpsimd.collective_compute(
                                    "AllReduce", OP.add,
                                    replica_groups=[list(range(N_CORES))],
                                    ins=[cin.opt()], outs=[cout.opt()])

                    # remaining pairs: kv(hp+1) then state(hp)
                    for hp in range(4):
                        if hp + 2 < 4:
                            kv_weights(hp + 2)
                        if hp + 1 < 4:
                            for t in range(NT):
                                kv_mm(hp + 1, t)
                        state_part(hp)

            # ============== C: q + hub -> qhatT8; absb + mt8 ==============
            with (
                tc.tile_pool(name=P("mtp"), bufs=1) as mtp,
                tc.tile_pool(name=P("mtw"), bufs=4) as mtw,
            ):
              with (
                tc.tile_pool(name=P("hubq"), bufs=4) as hubq,
                tc.tile_pool(name=P("ps_q"), bufs=2, space="PSUM") as ps_q,
                tc.tile_pool(name=P("ps_wpc"), bufs=2, space="PSUM") as ps_wpc,
              ):
                mt8 = [mtp.tile([128, 4, HD], F8, name=f"mt8_{rep}_{k}")
                       for k in range(H)]
                zpad = mtw.tile([128, HD], BF16, tag="zpad", bufs=1)
                nc.vector.memset(zpad[:], 0.0)
                for k in range(H):
                    nc.vector.tensor_copy(mt8[k][:, 3, :], zpad[:])

                def absb_build():
                    wsum16 = mtw.tile([1, 8], BF16, tag="ws16")
                    nc.sync.dma_start(wsum16[:], cc_out_a[CC_HALF:CC_HALF + 8])
                    ab = mtw.tile([1, 16], F32, tag="ab")
                    nc.vector.tensor_scalar_mul(ab[:, 0:8], gates_sb[:, 0:8],
                                                E_WP)
                    wsum = mtw.tile([1, 8], F32, tag="ws")
                    nc.vector.tensor_copy(wsum[:], wsum16[:])
                    nc.vector.tensor_scalar_mul(ab[:, 8:16], wsum[:], E_BA)
                    abr = mtw.tile([1, 16], F32R, tag="abr")
                    nc.vector.tensor_copy(abr[:], ab[:])
                    abt = ps_wpc.tile([128, D], F32, tag="wp")
                    nc.tensor.matmul(abt[:, 0:16], ones_row_r[:], abr[:])
                    nc.scalar.copy(absb[:], abt[:, 0:16])

                def mt_build(h):
                    st_bf = mtw.tile([128, 3, HD], BF16, tag="stbf",
                                     name=f"stbf_{rep}_{h}")
                    cc_t = cc_out_a if h < 4 else cc_out_b
                    base = (h % 4) * HD * HD
                    nc.sync.dma_start(
                        st_bf[:],
                        cc_t[base:base + HD * HD]
                        .rearrange("(e p f) -> p e f", e=3, p=128))
                    mm = mtw.tile([128, 3, HD], BF16, tag="memt",
                                  name=f"memt_{rep}_{h}")
                    nc.sync.dma_start(
                        mm[:], memT_d[h].rearrange("(e p) f -> p e f", p=128))
                    tmp = mtw.tile([128, 3, HD], BF16, tag="mtmp")
                    nc.vector.tensor_scalar_mul(tmp[:], st_bf[:],
                                                absb[:, 8 + h:9 + h])
                    mtbf = mtw.tile([128, 3, HD], BF16, tag="mtbf")
                    nc.vector.tensor_add(mtbf[:], mm[:], tmp[:])
                    nc.scalar.activation(mt8[h][:, 0:3, :], mtbf[:], AF.Copy,
                                         scale=1.0)
                    if DEBUG_DUMPS and h == 0:
                        nc.sync.dma_start(dbg["stbf0"].ap(), st_bf[:])
                        nc.sync.dma_start(dbg["mt80"].ap(), mt8[0][:])

                def q_chunk(sc):
                    jq, iq = sc // 2, sc % 2
                    qps = ps_q.tile([128, 1024], F32, tag="q")
                    for nh in range(2):
                        for j in range(3):
                            nc.tensor.matmul(
                                qps[:, nh * 512:(nh + 1) * 512],
                                w8q[j][:, :, sc * 128:(sc + 1) * 128],
                                xnT8[j][:, :, nh * 512:(nh + 1) * 512],
                                start=(j == 0), stop=(j == 2),
                                perf_mode=PM.DoubleRow)
                    if not b_in_zero:
                        nc.vector.tensor_scalar_add(qps[:], qps[:],
                                                    bq_sc[:, sc:sc + 1])
                    rbf = hubq.tile([128, 1024], BF16, tag="rq")
                    nc.scalar.activation(rbf[:], qps[:], AF.Relu, scale=C1)
                    nc.vector.scalar_tensor_tensor(
                        qhatT8[jq][:, iq, :], rbf[:], C2, rbf[:],
                        OP.add, OP.mult)

                def wprime(h):
                    wo = mtw.tile([128, 4, D], F8, tag="wo",
                                  name=f"wo_{rep}_{h}")
                    nc.sync.dma_start(wo[:], wo8_d[h])
                    m8 = mt8[h]
                    for dq in range(3):
                        pwp = ps_wpc.tile([128, D], F32, tag="wp")
                        for j in range(2):
                            stat = m8[:, 2 * j:2 * j + 2,
                                      dq * 128:(dq + 1) * 128]
                            nc.tensor.matmul(pwp[:, 0:512], stat,
                                             wo[:, 2 * j:2 * j + 2, 0:512],
                                             start=(j == 0), stop=(j == 1),
                                             perf_mode=PM.DoubleRow)
                            nc.tensor.matmul(pwp[:, 512:768], stat,
                                             wo[:, 2 * j:2 * j + 2, 512:768],
                                             start=(j == 0), stop=(j == 1),
                                             perf_mode=PM.DoubleRow)
                        sc = 3 * h + dq
                        dst = Wp8[sc // 2][:, sc % 2, :]
                        if (h + dq) % 2 == 0:
                            nc.scalar.activation(dst, pwp[:], AF.Copy,
                                                 scale=absb[:, h:h + 1])
                        else:
                            nc.vector.tensor_scalar_mul(dst, pwp[:],
                                                        absb[:, h:h + 1])

                for sc in range(8):
                    q_chunk(sc)
                absb_build()
                for h in range(H):
                    q_chunk(8 + 2 * h)
                    q_chunk(9 + 2 * h)
                    mt_build(h)
                    wprime(h)

            if DEBUG_DUMPS:
                nc.sync.dma_start(dbg["qh0"].ap(), qhatT8[0][:])
                nc.sync.dma_start(dbg["wp0"].ap(), Wp8[0][:])
                nc.sync.dma_start(dbg["gates"].ap(), gates_sb[:])
                nc.sync.dma_start(dbg["absb"].ap(), absb[:])
            # residue sigmoid + blend scalars
            nc.scalar.activation(residue[:], rlog_all[:], AF.Sigmoid,
                                 bias=bres_b[:], scale=1.0)
            nc.vector.tensor_scalar_mul(rs_col[:], residue[:], C_DEQ)
            nc.vector.tensor_scalar(onemr[:], residue[:], -1.0, 1.0,
                                    OP.mult, OP.add)

            if DEBUG_DUMPS:
                nc.sync.dma_start(dbg["residue"].ap(), residue[:])
            # ============== E: out projection + blend ==============
            with (
                tc.tile_pool(name=P("xe"), bufs=3) as xe,
                tc.tile_pool(name=P("ep"), bufs=4) as ep,
                tc.tile_pool(name=P("ps_o"), bufs=2, space="PSUM") as ps_o,
            ):
                if not b_out_zero:
                    bout_b = ep.tile([128, D], F32, tag="bout", bufs=1)
                    nc.sync.dma_start(bout_b[:],
                                      b_out_d.ap().partition_broadcast(128))
                for t in range(NT):
                    pos = ps_o.tile([128, D], F32, tag="o")
                    for jp in range(12):
                        stat = qhatT8[jp][:, :, t * 128:(t + 1) * 128]
                        nc.tensor.matmul(pos[:, 0:512], stat,
                                         Wp8[jp][:, :, 0:512],
                                         start=(jp == 0), stop=(jp == 11),
                                         perf_mode=PM.DoubleRow)
                        nc.tensor.matmul(pos[:, 512:768], stat,
                                         Wp8[jp][:, :, 512:768],
                                         start=(jp == 0), stop=(jp == 11),
                                         perf_mode=PM.DoubleRow)
                    xfull = xe.tile([128, D], F32, tag="xe")
                    nc.scalar.dma_start(xfull[:], x_d[t * 128:(t + 1) * 128, :])
                    u = xe.tile([128, D], F32, tag="u")
                    nc.scalar.activation(u[:], xfull[:], AF.Copy,
                                         scale=onemr[:, t:t + 1])
                    if not b_out_zero:
                        ub = xe.tile([128, D], F32, tag="ub")
                        nc.vector.tensor_scalar_mul(ub[:], bout_b[:],
                                                    residue[:, t:t + 1])
                        nc.vector.tensor_add(u[:], u[:], ub[:])
                    ot = ep.tile([128, D], F32, tag="ot")
                    nc.vector.scalar_tensor_tensor(
                        ot[:], pos[:], rs_col[:, t:t + 1], u[:],
                        OP.mult, OP.add)
                    nc.sync.dma_start(out_d[t * 128:(t + 1) * 128, :], ot[:])

    nc.compile()
    return nc


_PROGRAM_CACHE = {}


def _get_program(key):
    if key not in _PROGRAM_CACHE:
        _PROGRAM_CACHE[key] = build_program(*key)
    return _PROGRAM_CACHE[key]


def _q8(a, scale):
    y = np.asarray(a, np.float32) * scale
    y = np.clip(y, -224.0, 224.0)
    return y.astype(ml_dtypes.float8_e4m3)


def kernel(x, memory, ln_g, ln_b, w_in, b_in, w_out, b_out,
           w_rg, b_rg, w_wg, b_wg, w_res, b_res):
    x = np.ascontiguousarray(np.asarray(x, dtype=np.float32))
    memory = np.asarray(memory, dtype=np.float32)
    ln_g = np.asarray(ln_g, dtype=np.float32)
    ln_b = np.asarray(ln_b, dtype=np.float32)
    w_in = np.ascontiguousarray(np.asarray(w_in, dtype=np.float32))
    b_in = np.asarray(b_in, dtype=np.float32)
    w_out = np.asarray(w_out, dtype=np.float32)
    b_out = np.asarray(b_out, dtype=np.float32)
    w_rg = np.asarray(w_rg, dtype=np.float32)
    b_rg = np.asarray(b_rg, dtype=np.float32)
    w_wg = np.asarray(w_wg, dtype=np.float32)
    b_wg = np.asarray(b_wg, dtype=np.float32)
    w_res = np.asarray(w_res, dtype=np.float32)
    b_res = np.asarray(b_res, dtype=np.float32)

    ln_trivial = bool(np.all(ln_g == 1.0) and np.all(ln_b == 0.0))
    b_in_zero = bool(np.all(b_in == 0.0))
    b_out_zero = bool(np.all(b_out == 0.0))

    nc = _get_program((ln_trivial, b_in_zero, b_out_zero))

    # ---- pack w8 image: [3 j][128 p][2 i][WCOLS], d = 256j + 128i + p ----
    cols = np.empty((D, WCOLS), np.float32)
    for hp in range(4):
        c0 = hp * KVB
        cols[:, c0:c0 + 768] = w_in[:, S + 2 * hp * HD:S + (2 * hp + 2) * HD]
        cols[:, c0 + 768:c0 + 1536] = \
            w_in[:, 2 * S + 2 * hp * HD:2 * S + (2 * hp + 2) * HD]
    cols[:, QOFF:QOFF + S] = w_in[:, 0:S]
    cols[:, GOFF:GOFF + 8] = w_rg
    cols[:, GOFF + 8:GOFF + 16] = w_wg
    w8 = _q8(cols, S_W).reshape(3, 2, 128, WCOLS).transpose(0, 2, 1, 3)
    w8 = np.ascontiguousarray(w8)

    # ---- wo8: per head [128 p][4 chunk][D], e' = 128*chunk + p; chunk3 = 0
    wo8 = np.zeros((H, 128, 4, D), ml_dtypes.float8_e4m3)
    wo_q = _q8(w_out, S_WO).reshape(H, 3, 128, D)
    wo8[:, :, 0:3, :] = wo_q.transpose(0, 2, 1, 3)
    wo8 = np.ascontiguousarray(wo8)

    memT = np.ascontiguousarray(
        memory.transpose(0, 2, 1) * S_M).astype(ml_dtypes.bfloat16)
    gb = np.concatenate([b_rg, b_wg]).astype(np.float32)

    shared = {
        "w8": w8, "wo8": wo8, "memT": memT, "gb": gb,
        "w_res": w_res, "b_res": b_res,
    }
    if not ln_trivial:
        shared["ln_g"] = ln_g
        shared["ln_b"] = ln_b
    if not b_in_zero:
        shared["b_in"] = b_in * (S_X * S_W)
    if not b_out_zero:
        shared["b_out"] = b_out
    in_maps = [{"x": x[b], **shared} for b in range(N_CORES)]
    res = run_bass_kernel_spmd(nc, in_maps, list(range(N_CORES)))
    return np.stack([res.results[b]["out"] for b in range(N_CORES)], axis=0)
